# revision 11
# baseline (speedup 1.0000x reference)
"""GATConvBlock (GAT attention + BatchNorm + LeakyReLU) on 8 Trainium2
NeuronCores. Self-contained: host-side edge scheduling + Bass/Tile program +
SPMD execution via concourse.

Strategy: dst-sharded graph parallelism, fp16 datapath. Each core owns 12500
dst nodes; x is rotated per core so its shard is table rows 0:12500. Phase 1
computes a full per-node table row [h fp16 x128 | a_src f32 x4 | a_dst f32 x4]
(512B stride) with interleaved 4-tile loads. Phase 2 processes edges grouped
by (dst-tile, src-range): int16 dma_gather of source rows, membership matrices
built as batched vector ops (interval masks from host-precomputed start/end
columns; one-hot via batched is_equal), per-edge a_dst via small PE matmuls,
segment softmax + aggregation via membership matmuls accumulating in PSUM.
BatchNorm stats via ones-matmuls into one PSUM tile + AllReduce."""
import numpy as np

import concourse.bass as bass
import concourse.bacc as bacc
import concourse.tile as tile
from concourse import mybir
from concourse.bass_utils import run_bass_kernel_spmd

# ---- walrus compat: split multi-wait sync_info (this toolchain rejects >1) ----
from concourse import mybir as _mb
from concourse.tile import TileContext as _TC
from concourse.vector_clock import ScopedClock as _SC

_MAX_WAITS = 1


def _patched_drain_and_barrier(self, tick_clock, wait_clock):
    drain_inst = self.nc.sync.drain()
    wait_clock.add_sem_waits(drain_inst.ins, _SC({None: tick_clock.global_clock}))
    si = drain_inst.ins.sync_info
    waits = list(si.on_wait or [])
    if len(waits) > _MAX_WAITS:
        si.on_wait = waits[:_MAX_WAITS]
        for w in waits[_MAX_WAITS:]:
            n = self.nc.sync.nop()
            n.ins.sync_info = _mb.SyncInfo(on_wait=[w], on_update=[])
        self.nc.sync.drain()
    self.nc.all_engine_barrier()
    popped = self.nc._tile_sem_poison_stack.pop()
    assert popped is self._sem_poison
    self.nc.clear_and_free_semaphores(list(self.sems.allocated().values()))
    self.nc.all_engine_barrier()


_orig_commit = _TC._commit_instruction


def _patched_commit(self, inst, lazy_reg_writes=True):
    si = inst.sync_info
    if si is not None and si.on_wait and len(si.on_wait) > _MAX_WAITS and \
            inst.engine != _mb.EngineType.Unassigned:
        waits = list(si.on_wait)
        si.on_wait = waits[:_MAX_WAITS]
        eng = self.nc.engines[inst.engine]
        for w in waits[_MAX_WAITS:]:
            n = eng.nop()
            n.ins.sync_info = _mb.SyncInfo(on_wait=[w], on_update=[])
    return _orig_commit(self, inst, lazy_reg_writes)


if getattr(_TC, "_gat_patched", False) is False:
    _TC._drain_and_barrier = _patched_drain_and_barrier
    _TC._commit_instruction = _patched_commit
    _TC._gat_patched = True
# ---- end walrus compat ----


f32 = mybir.dt.float32
f16 = mybir.dt.float16
i16 = mybir.dt.int16
AF = mybir.ActivationFunctionType
ALU = mybir.AluOpType

P = 128
RANGE = 32768
NEG = 0.2
BN_EPS = 1e-5
GATHER_MAX = 896


def build_schedule(src_g, dst_g, n_nodes, n_cores):
    shard = n_nodes // n_cores
    n_tiles = (shard + P - 1) // P
    n_ranges = (n_nodes + RANGE - 1) // RANGE

    order = np.argsort(dst_g, kind="stable")
    src_s = src_g[order]
    dst_s = dst_g[order]
    core_bounds = np.searchsorted(dst_s, np.arange(n_cores + 1) * shard)

    counts = np.zeros((n_cores, n_tiles, n_ranges), np.int64)
    segs = [[[None] * n_ranges for _ in range(n_tiles)] for _ in range(n_cores)]
    for c in range(n_cores):
        lo, hi = core_bounds[c], core_bounds[c + 1]
        s = (src_s[lo:hi] - c * shard) % n_nodes
        d = dst_s[lo:hi] - c * shard
        t_id = d // P
        r_id = s // RANGE
        o2 = np.lexsort((d, r_id, t_id))
        s, d, t_id, r_id = s[o2], d[o2], t_id[o2], r_id[o2]
        key = t_id * n_ranges + r_id
        cnt = np.bincount(key, minlength=n_tiles * n_ranges).reshape(
            n_tiles, n_ranges)
        counts[c] = cnt
        offs = np.concatenate([[0], np.cumsum(cnt.ravel())])
        for t in range(n_tiles):
            for r in range(n_ranges):
                k = t * n_ranges + r
                sl = slice(offs[k], offs[k + 1])
                segs[c][t][r] = (s[sl], d[sl] - t * P)

    caps = counts.max(axis=0)
    caps = ((caps + P - 1) // P) * P
    cap_max = int(caps.max())
    assert cap_max <= 2048, cap_max

    G = 4
    n_grp2 = (n_tiles + G - 1) // G
    groups = [list(range(g * G, min((g + 1) * G, n_tiles)))
              for g in range(n_grp2)]
    o16 = np.zeros((n_tiles, n_ranges), np.int64)
    odl = np.zeros((n_tiles, n_ranges), np.int64)
    goff = np.zeros((n_tiles, n_ranges), np.int64)   # chunk offset in group buf
    gcap = np.zeros((n_grp2, n_ranges), np.int64)    # idx count per (group, r)
    go16 = np.zeros((n_grp2, n_ranges), np.int64)    # idx16 col of group block
    acc16 = accdl = 0
    for gi, ts in enumerate(groups):
        for r in range(n_ranges):
            go16[gi, r] = acc16
            coff = 0
            for t in ts:
                o16[t, r] = acc16
                goff[t, r] = coff
                acc16 += int(caps[t, r]) // 16
                coff += int(caps[t, r]) // P
            gcap[gi, r] = coff * P
    for t in range(n_tiles):
        for r in range(n_ranges):
            odl[t, r] = accdl
            accdl += int(caps[t, r]) // P
    idxcols, dlcols = acc16, accdl

    per_core = []
    for c in range(n_cores):
        idx_arr = np.zeros((16, idxcols), np.int16)
        dl_arr = np.full((P, dlcols), -1.0, np.float16)
        for t in range(n_tiles):
            for r in range(n_ranges):
                cap = int(caps[t, r])
                if cap == 0:
                    continue
                sseg, dseg = segs[c][t][r]
                n = len(sseg)
                i16v = np.zeros(cap, np.int16)
                i16v[:n] = (sseg - r * RANGE).astype(np.int16)
                dlv = np.full(cap, -1.0, np.float16)
                dlv[:n] = dseg.astype(np.float16)
                idx_arr[:, int(o16[t, r]): int(o16[t, r]) + cap // 16] = \
                    i16v.reshape(cap // 16, 16).T
                dl_arr[:, int(odl[t, r]): int(odl[t, r]) + cap // P] = \
                    dlv.reshape(cap // P, P).T
        per_core.append({
            "g_idx": np.tile(idx_arr, (8, 1)),
            "g_dl": dl_arr,
        })
    sched = {"caps": caps, "o16": o16, "odl": odl, "idxcols": idxcols,
             "dlcols": dlcols, "n_tiles": n_tiles, "n_ranges": n_ranges,
             "shard": shard, "cap_max": cap_max, "n_nodes": n_nodes,
             "groups": groups, "goff": goff, "gcap": gcap, "go16": go16}
    return sched, per_core


def build_program(sched):
    n_nodes = sched["n_nodes"]
    shard = sched["shard"]
    n_tiles, n_ranges = sched["n_tiles"], sched["n_ranges"]
    caps, o16, odl = sched["caps"], sched["o16"], sched["odl"]
    cap_max = sched["cap_max"]
    out_ch = P

    nc = bacc.Bacc("TRN2", target_bir_lowering=False, num_devices=8,
                   num_swdge_queues=4)
    x_in = nc.dram_tensor("x", [n_nodes, P], f16, kind="ExternalInput")
    w_in = nc.dram_tensor("w", [P, P], f32, kind="ExternalInput")
    asrc_in = nc.dram_tensor("att_src", [4, 32], f32, kind="ExternalInput")
    adst_in = nc.dram_tensor("att_dst", [4, 32], f32, kind="ExternalInput")
    gamma_in = nc.dram_tensor("gamma", [1, out_ch], f32, kind="ExternalInput")
    beta_in = nc.dram_tensor("beta", [1, out_ch], f32, kind="ExternalInput")
    gidx_in = nc.dram_tensor("g_idx", [P, sched["idxcols"]], i16,
                             kind="ExternalInput")
    gdl_in = nc.dram_tensor("g_dl", [P, sched["dlcols"]], f16,
                            kind="ExternalInput")
    iota_in = nc.dram_tensor("iota", [P, cap_max], f16, kind="ExternalInput")
    ident_in = nc.dram_tensor("ident", [P, P], f32, kind="ExternalInput")
    out_fin = nc.dram_tensor("out", [shard, out_ch], f32, kind="ExternalOutput")

    with tile.TileContext(nc) as tc:
        with tc.tile_pool(name="dram", bufs=1, space="DRAM") as drp, \
             tc.tile_pool(name="const", bufs=1) as cs:
            table = drp.tile([n_nodes, P], f32)
            bn_in = drp.tile([1, 2 * out_ch], f32)
            bn_out = drp.tile([1, 2 * out_ch], f32)

            # ---------------- constants ----------------
            identf = cs.tile([P, P], f32)
            nc.sync.dma_start(out=identf[:], in_=ident_in[:])
            identh = cs.tile([P, P], f16)
            nc.vector.tensor_copy(out=identh[:], in_=identf[:])
            iota_sb = cs.tile([P, cap_max], f16)
            nc.sync.dma_start(out=iota_sb[:], in_=iota_in[:])
            gidx_sb = cs.tile([P, sched["idxcols"]], i16)
            nc.sync.dma_start(out=gidx_sb[:], in_=gidx_in[:])
            gdl_sb = cs.tile([P, sched["dlcols"]], f16)
            nc.sync.dma_start(out=gdl_sb[:], in_=gdl_in[:])
            alpha_c = cs.tile([P, 1], f32)
            nc.vector.memset(alpha_c[:], NEG)
            ones_h = cs.tile([P, 1], f16)
            nc.vector.memset(ones_h[:], 1.0)
            ones1h = cs.tile([1, P], f16)
            nc.vector.memset(ones1h[:], 1.0)
            eps_c = cs.tile([1, 1], f32)
            nc.vector.memset(eps_c[:], BN_EPS)
            o_all = cs.tile([P, n_tiles, P], f16)

            # ---------------- phase 0: rhs_big = [W | W@A] fp16 ----------------
            rhs_big = cs.tile([P, P + 8], f16)
            with tc.tile_pool(name="p0", bufs=1) as p0, \
                 tc.tile_pool(name="p0ps", bufs=2, space="PSUM") as p0ps:
                wf = p0.tile([P, P], f32)
                nc.sync.dma_start(out=wf[:], in_=w_in[:])
                wh = p0.tile([P, P], f16)
                nc.vector.tensor_copy(out=wh[:], in_=wf[:])
                A_sb = p0.tile([P, 8], f32)
                nc.vector.memset(A_sb[:], 0.0)
                for hd in range(4):
                    nc.sync.dma_start(
                        out=A_sb[hd * 32:(hd + 1) * 32, hd:hd + 1],
                        in_=asrc_in[hd:hd + 1, :].rearrange("a b -> b a"))
                    nc.sync.dma_start(
                        out=A_sb[hd * 32:(hd + 1) * 32, 4 + hd:5 + hd],
                        in_=adst_in[hd:hd + 1, :].rearrange("a b -> b a"))
                Ah = p0.tile([P, 8], f16)
                nc.vector.tensor_copy(out=Ah[:], in_=A_sb[:])
                WT_ps = p0ps.tile([P, P], f16)
                nc.tensor.transpose(out=WT_ps[:], in_=wh[:], identity=identh[:])
                WT = p0.tile([P, P], f16)
                nc.scalar.copy(out=WT[:], in_=WT_ps[:])
                WA_ps = p0ps.tile([P, 8], f32)
                nc.tensor.matmul(out=WA_ps[:], lhsT=WT[:], rhs=Ah[:],
                                 start=True, stop=True)
                nc.vector.tensor_copy(out=rhs_big[:, 0:P], in_=wh[:])
                nc.vector.tensor_copy(out=rhs_big[:, P:P + 8], in_=WA_ps[:])

            # ---------------- phase 1: table build (xbar-transposed x) ----
            n_grp = (n_nodes + 511) // 512
            with tc.tile_pool(name="p1", bufs=5) as p1, \
                 tc.tile_pool(name="p1h", bufs=3, space="PSUM") as p1h:
                for g in range(n_grp):
                    r0 = g * 512
                    rows = min(512, n_nodes - r0)
                    xT4 = p1.tile([P, 512], f16, tag="xT")
                    nc.sync.dma_start(out=xT4[:, 0:rows],
                                      in_=x_in[r0:r0 + rows, :],
                                      transpose=True)
                    stg = p1.tile([P, 4, 74], f32, tag="stg")
                    ones_v = stg[:, :, 0:66].bitcast(f16).rearrange(
                        "p j (h c) -> p j h c", h=4)[:, :, :, 32:33]
                    nc.vector.memset(ones_v, 1.0)
                    nsub = (rows + P - 1) // P
                    for j in range(nsub):
                        ptn = min(P, rows - j * P)
                        h_ps = p1h.tile([P, P + 8], f32, tag="hp")
                        nc.tensor.matmul(out=h_ps[:ptn],
                                         lhsT=xT4[:, j * P:j * P + ptn],
                                         rhs=rhs_big[:], start=True, stop=True)
                        nc.vector.tensor_copy(
                            out=stg[:ptn, j, 0:66].bitcast(f16).rearrange(
                                "p (h c) -> p h c", h=4)[:, :, 0:32],
                            in_=h_ps[:ptn, 0:P].rearrange(
                                "p (h c) -> p h c", h=4))
                        nc.scalar.copy(out=stg[:ptn, j, 66:74],
                                       in_=h_ps[:ptn, P:P + 8])
                    if rows == 512:
                        nc.sync.dma_start(
                            out=table[r0:r0 + rows].rearrange(
                                "(j p) c -> p j c", j=4)[:, :, 0:74],
                            in_=stg[:])
                    else:
                        for j in range(nsub):
                            ptn = min(P, rows - j * P)
                            nc.sync.dma_start(
                                out=table[r0 + j * P:r0 + j * P + ptn, 0:74],
                                in_=stg[:ptn, j, :])

            # ---------------- phase 2: edges ----------------
            gq = [0]
            groups = sched["groups"]
            goff, gcap, go16 = sched["goff"], sched["gcap"], sched["go16"]
            with tc.tile_pool(name="gb", bufs=2) as gbp, \
                 tc.tile_pool(name="wk", bufs=5) as wk, \
                 tc.tile_pool(name="tl", bufs=3) as tlp, \
                 tc.tile_pool(name="agg", bufs=2, space="PSUM") as aggp, \
                 tc.tile_pool(name="aep", bufs=2, space="PSUM") as aep, \
                 tc.tile_pool(name="trp", bufs=2, space="PSUM") as trp, \
                 tc.tile_pool(name="stp", bufs=1, space="PSUM") as stp:
                stats_ps = stp.tile([1, 2 * out_ch], f32)
                for gi, ts in enumerate(groups):
                    # one gather per (group, range)
                    gtiles = {}
                    for r in range(n_ranges):
                        gc = int(gcap[gi, r])
                        if gc == 0:
                            continue
                        base = r * RANGE
                        end = min(base + RANGE, n_nodes)
                        gg = gbp.tile([P, gc // P, P], f32, tag=f"g{r}",
                                      name=f"g_{gi}_{r}")
                        off = 0
                        while off < gc:
                            piece = min(GATHER_MAX, gc - off)
                            nc.gpsimd.dma_gather(
                                out_ap=gg[:, off // P:(off + piece) // P, :],
                                in_ap=table[base:end, :],
                                idxs_ap=gidx_sb[:, int(go16[gi, r]) + off // 16:
                                                int(go16[gi, r]) + (off + piece) // 16],
                                num_idxs=piece,
                                num_idxs_reg=piece,
                                elem_size=P,
                                queue_num=gq[0] % 4,
                            )
                            gq[0] += 1
                            off += piece
                        gtiles[r] = gg
                    for t in ts:
                        d0 = t * P
                        pt = min(P, shard - d0)
                        adf = tlp.tile([P, 4], f32, tag="adf")
                        nc.sync.dma_start(out=adf[:pt],
                                          in_=table[d0:d0 + pt, 70:74])
                        adh = tlp.tile([P, 4], f16, tag="adh")
                        nc.vector.memset(adh[:], 0.0)
                        nc.vector.tensor_copy(out=adh[:pt], in_=adf[:pt])

                        psum_t = aggp.tile([P, out_ch + 4], f32, tag="agg",
                                           name=f"agg_{t}")
                        n_chunks = sum(int(caps[t, r]) // P
                                       for r in range(n_ranges))
                        done = 0
                        for r in range(n_ranges):
                            cap = int(caps[t, r])
                            if cap == 0:
                                continue
                            ch = cap // P
                            c0 = int(goff[t, r])
                            g = gtiles[r][:, c0:c0 + ch, :]
                            # Mc one-hot [e, d] batched over chunks
                            Mc = wk.tile([P, ch, P], f16, tag="mc",
                                         name=f"c_{t}_{r}")
                            dls = gdl_sb[:, int(odl[t, r]):int(odl[t, r]) + ch]
                            dl3 = bass.AP(tensor=dls.tensor, offset=dls.offset,
                                          ap=[dls.ap[0], dls.ap[1], [0, P]])
                            io = iota_sb[:, 0:P]
                            io3 = bass.AP(tensor=io.tensor, offset=io.offset,
                                          ap=[io.ap[0], [0, ch], io.ap[1]])
                            nc.vector.tensor_tensor(out=Mc[:], in0=dl3,
                                                    in1=io3, op=ALU.is_equal)
                            # McT chunks via PE transpose (2 per copy)
                            ae_ps = aep.tile([P, ch, 4], f32, tag="ae",
                                             name=f"ae_{t}_{r}")
                            for k2 in range(0, ch, 2):
                                kn = min(2, ch - k2)
                                mt_ps = trp.tile([P, 2 * P], f16, tag="tr")
                                for kk in range(kn):
                                    nc.tensor.transpose(
                                        out=mt_ps[:, kk * P:(kk + 1) * P],
                                        in_=Mc[:, k2 + kk, :],
                                        identity=identh[:])
                                mct = wk.tile([P, 2 * P], f16, tag="mcts")
                                nc.scalar.copy(out=mct[:, 0:kn * P],
                                               in_=mt_ps[:, 0:kn * P])
                                for kk in range(kn):
                                    nc.tensor.matmul(
                                        out=ae_ps[:, k2 + kk, :],
                                        lhsT=mct[:, kk * P:(kk + 1) * P],
                                        rhs=adh[:], start=True, stop=True)
                            # scores -> exp (leaky on V, exp on scalar)
                            s3 = wk.tile([P, ch, 4], f32, tag="s3",
                                         name=f"s_{t}_{r}")
                            nc.vector.tensor_tensor(out=s3[:],
                                                    in0=g[:, :, 66:70],
                                                    in1=ae_ps[:], op=ALU.add)
                            lr3 = wk.tile([P, ch, 4], f32, tag="lr",
                                          name=f"l_{t}_{r}")
                            nc.vector.scalar_tensor_tensor(
                                out=lr3[:], in0=s3[:], scalar=NEG,
                                in1=s3[:], op0=ALU.mult, op1=ALU.max)
                            ex3 = wk.tile([P, ch, 4], f16, tag="ex",
                                          name=f"e_{t}_{r}")
                            nc.scalar.activation(out=ex3[:], in_=lr3[:],
                                                 func=AF.Exp)
                            # me = [h*ex | ex] via ones-interleaved rows
                            me = wk.tile([P, ch, out_ch + 4], f16, tag="me",
                                         name=f"m_{t}_{r}")
                            hv = g[:, :, 0:66].bitcast(f16)
                            ex_b = bass.AP(
                                tensor=ex3.tensor, offset=ex3[:].offset,
                                ap=[ex3[:].ap[0], ex3[:].ap[1], ex3[:].ap[2],
                                    [0, 33]])
                            nc.vector.tensor_tensor(
                                out=me[:].rearrange(
                                    "p a (h c) -> p a h c", h=4),
                                in0=hv.rearrange("p a (h c) -> p a h c", h=4),
                                in1=ex_b, op=ALU.mult)
                            for k in range(ch):
                                nc.tensor.matmul(out=psum_t[:],
                                                 lhsT=Mc[:, k, :],
                                                 rhs=me[:, k, :],
                                                 start=(done == 0),
                                                 stop=(done == n_chunks - 1))
                                done += 1

                        # ---- per-tile epilogue ----
                        pv = psum_t[:pt].rearrange("p (h c) -> p h c", h=4)
                        den = tlp.tile([P, 4], f32, tag="den")
                        nc.vector.tensor_scalar(out=den[:pt],
                                                in0=pv[:, :, 32:33],
                                                scalar1=1e-16, scalar2=None,
                                                op0=ALU.add)
                        rden = tlp.tile([P, 4], f32, tag="rden")
                        nc.vector.reciprocal(out=rden[:pt], in_=den[:pt])
                        rv = rden[:pt]
                        rd_b = bass.AP(tensor=rv.tensor, offset=rv.offset,
                                       ap=[rv.ap[0], rv.ap[1], [0, 32]])
                        nc.vector.tensor_tensor(
                            out=o_all[:pt, t, :].rearrange("p (h c) -> p h c", h=4),
                            in0=pv[:, :, 0:32],
                            in1=rd_b, op=ALU.mult)
                        sq = tlp.tile([P, P], f16, tag="sq")
                        nc.vector.tensor_tensor(out=sq[:pt], in0=o_all[:pt, t, :],
                                                in1=o_all[:pt, t, :], op=ALU.mult)
                        nc.tensor.matmul(out=stats_ps[0:1, 0:P],
                                         lhsT=ones_h[:pt], rhs=o_all[:pt, t, :],
                                         start=(t == 0), stop=(t == n_tiles - 1))
                        nc.tensor.matmul(out=stats_ps[0:1, P:2 * P],
                                         lhsT=ones_h[:pt], rhs=sq[:pt],
                                         start=(t == 0), stop=(t == n_tiles - 1))

                # ---------------- BN stats allreduce ----------------
                with tc.tile_pool(name="bn", bufs=1) as bnp, \
                     tc.tile_pool(name="bnps", bufs=1, space="PSUM") as bnps:
                    stats_sb = bnp.tile([1, 2 * out_ch], f32)
                    nc.scalar.copy(out=stats_sb[:], in_=stats_ps[:])
                    nc.sync.dma_start(out=bn_in[:], in_=stats_sb[:])
                    nc.gpsimd.collective_compute(
                        "AllReduce", ALU.add,
                        replica_groups=[list(range(8))],
                        ins=[bn_in[:]], outs=[bn_out[:]],
                    )
                    tots = bnp.tile([1, 2 * out_ch], f32)
                    nc.sync.dma_start(out=tots[:], in_=bn_out[:])
                    gam = bnp.tile([1, out_ch], f32)
                    nc.sync.dma_start(out=gam[:], in_=gamma_in[:])
                    bet = bnp.tile([1, out_ch], f32)
                    nc.sync.dma_start(out=bet[:], in_=beta_in[:])
                    mu = bnp.tile([1, out_ch], f32)
                    nc.vector.tensor_scalar(out=mu[:], in0=tots[:, 0:out_ch],
                                            scalar1=1.0 / n_nodes,
                                            scalar2=None, op0=ALU.mult)
                    ex2 = bnp.tile([1, out_ch], f32)
                    nc.vector.tensor_scalar(out=ex2[:], in0=tots[:, out_ch:],
                                            scalar1=1.0 / n_nodes,
                                            scalar2=None, op0=ALU.mult)
                    musq = bnp.tile([1, out_ch], f32)
                    nc.vector.tensor_tensor(out=musq[:], in0=mu[:], in1=mu[:],
                                            op=ALU.mult)
                    var = bnp.tile([1, out_ch], f32)
                    nc.vector.tensor_tensor(out=var[:], in0=ex2[:],
                                            in1=musq[:], op=ALU.subtract)
                    sd = bnp.tile([1, out_ch], f32)
                    nc.scalar.activation(out=sd[:], in_=var[:], func=AF.Sqrt,
                                         bias=eps_c[0:1, 0:1], scale=1.0)
                    rs = bnp.tile([1, out_ch], f32)
                    nc.vector.reciprocal(out=rs[:], in_=sd[:])
                    scale_r = bnp.tile([1, out_ch], f32)
                    nc.vector.tensor_tensor(out=scale_r[:], in0=rs[:],
                                            in1=gam[:], op=ALU.mult)
                    mshift = bnp.tile([1, out_ch], f32)
                    nc.vector.tensor_tensor(out=mshift[:], in0=mu[:],
                                            in1=scale_r[:], op=ALU.mult)
                    shift_r = bnp.tile([1, out_ch], f32)
                    nc.vector.tensor_tensor(out=shift_r[:], in0=bet[:],
                                            in1=mshift[:], op=ALU.subtract)
                    scsh = bnp.tile([1, 2 * out_ch], f16)
                    nc.vector.tensor_copy(out=scsh[:, 0:out_ch],
                                          in_=scale_r[:])
                    nc.vector.tensor_copy(out=scsh[:, out_ch:], in_=shift_r[:])
                    bc_ps = bnps.tile([P, 2 * out_ch], f32)
                    nc.tensor.matmul(out=bc_ps[:], lhsT=ones1h[:],
                                     rhs=scsh[:], start=True, stop=True)
                    scale_bc = cs.tile([P, out_ch], f32)
                    nc.vector.tensor_copy(out=scale_bc[:],
                                          in_=bc_ps[:, 0:out_ch])
                    shift_bc = cs.tile([P, out_ch], f32)
                    nc.scalar.copy(out=shift_bc[:], in_=bc_ps[:, out_ch:])

                # ---------------- phase 3: normalize ----------------
                with tc.tile_pool(name="p3", bufs=3) as p3:
                    for t in range(n_tiles):
                        d0 = t * P
                        pt = min(P, shard - d0)
                        y_t = p3.tile([P, out_ch], f32, tag="y3")
                        nc.vector.tensor_tensor(out=y_t[:pt],
                                                in0=o_all[:pt, t, :],
                                                in1=scale_bc[:pt],
                                                op=ALU.mult)
                        nc.vector.tensor_tensor(out=y_t[:pt], in0=y_t[:pt],
                                                in1=shift_bc[:pt], op=ALU.add)
                        z_t = p3.tile([P, out_ch], f32, tag="z3")
                        nc.scalar.activation(out=z_t[:pt], in_=y_t[:pt],
                                             func=AF.Prelu,
                                             alpha=alpha_c[:pt, 0:1])
                        nc.sync.dma_start(out=out_fin[d0:d0 + pt, :],
                                          in_=z_t[:pt])

    nc.compile()
    return nc


def run(x, edge_index, W, att_src, att_dst, bias, gamma, beta,
        n_cores=8, trace=False, tmpdir=None):
    n_nodes, in_ch = x.shape
    shard = n_nodes // n_cores

    src = np.asarray(edge_index[0], np.int64)
    dst = np.asarray(edge_index[1], np.int64)
    sched, per_core = build_schedule(src, dst, n_nodes, n_cores)
    nc = build_program(sched)

    x = np.asarray(x, np.float32)
    iota_np = np.tile(np.arange(sched["cap_max"], dtype=np.float16), (P, 1))
    in_maps = []
    for c in range(n_cores):
        in_maps.append({
            "x": np.roll(x, -c * shard, axis=0).astype(np.float16),
            "w": np.asarray(W, np.float32),
            "att_src": np.asarray(att_src, np.float32),
            "att_dst": np.asarray(att_dst, np.float32),
            "gamma": np.asarray(gamma, np.float32).reshape(1, -1),
            "beta": np.asarray(beta, np.float32).reshape(1, -1),
            "g_idx": per_core[c]["g_idx"],
            "g_dl": per_core[c]["g_dl"],
            "iota": iota_np,
            "ident": np.eye(P, dtype=np.float32),
        })
    res = run_bass_kernel_spmd(nc, in_maps, core_ids=list(range(n_cores)),
                               trace=trace, tmpdir=tmpdir)
    out = np.concatenate([res.results[c]["out"] for c in range(n_cores)],
                         axis=0)
    return out, res


def _install_ntff_hook():
    """Best-effort NTFF profile hook for trace mode (missing in this image)."""
    import sys, types
    try:
        import antenv.axon_hooks  # noqa: F401
        return
    except ImportError:
        pass
    try:
        from trn_agent_boot.trn_boot import _ntff_profile_via_ctypes
        hook = _ntff_profile_via_ctypes('/opt/axon/libaxon_pjrt.so')
    except Exception:
        hook = None
    mod = types.ModuleType("antenv.axon_hooks")
    mod.get_axon_ntff_profile_hook = lambda: hook
    mod.set_axon_ntff_profile_hook = lambda h: None
    sys.modules["antenv.axon_hooks"] = mod


def kernel(**inputs):
    x = np.asarray(inputs["x"], np.float32)
    edge_index = np.asarray(inputs["edge_index"])
    W = np.asarray(inputs["W"], np.float32)
    att_src = np.asarray(inputs["att_src"], np.float32)
    att_dst = np.asarray(inputs["att_dst"], np.float32)
    bias = np.asarray(inputs["bias"], np.float32)
    gamma = np.asarray(inputs["gamma"], np.float32)
    beta = np.asarray(inputs["beta"], np.float32)
    import os
    trace = bool(os.environ.get("GAT_TRACE"))
    if trace:
        _install_ntff_hook()
    tmpdir = os.environ.get("GAT_TRACE_DIR")
    out, res = run(x, edge_index, W, att_src, att_dst, bias, gamma, beta,
                   trace=trace, tmpdir=tmpdir)
    kernel.last_result = res
    return out


# revision 12
# speedup vs baseline: 1.2224x; 1.2224x over previous
"""GATConvBlock (GAT attention + BatchNorm + LeakyReLU) on 8 Trainium2
NeuronCores. Self-contained: host-side edge scheduling + Bass/Tile program +
SPMD execution via concourse.

Strategy: dst-sharded graph parallelism, fp16 datapath. Each core owns 12500
dst nodes; x is rotated per core so its shard is table rows 0:12500. Phase 1
computes a full per-node table row [h fp16 x128 | a_src f32 x4 | a_dst f32 x4]
(512B stride) with interleaved 4-tile loads. Phase 2 processes edges grouped
by (dst-tile, src-range): int16 dma_gather of source rows, membership matrices
built as batched vector ops (interval masks from host-precomputed start/end
columns; one-hot via batched is_equal), per-edge a_dst via small PE matmuls,
segment softmax + aggregation via membership matmuls accumulating in PSUM.
BatchNorm stats via ones-matmuls into one PSUM tile + AllReduce."""
import numpy as np

import concourse.bass as bass
import concourse.bacc as bacc
import concourse.tile as tile
from concourse import mybir
from concourse.bass_utils import run_bass_kernel_spmd

# ---- walrus compat: split multi-wait sync_info (this toolchain rejects >1) ----
from concourse import mybir as _mb
from concourse.tile import TileContext as _TC
from concourse.vector_clock import ScopedClock as _SC

_MAX_WAITS = 1


def _patched_drain_and_barrier(self, tick_clock, wait_clock):
    drain_inst = self.nc.sync.drain()
    wait_clock.add_sem_waits(drain_inst.ins, _SC({None: tick_clock.global_clock}))
    si = drain_inst.ins.sync_info
    waits = list(si.on_wait or [])
    if len(waits) > _MAX_WAITS:
        si.on_wait = waits[:_MAX_WAITS]
        for w in waits[_MAX_WAITS:]:
            n = self.nc.sync.nop()
            n.ins.sync_info = _mb.SyncInfo(on_wait=[w], on_update=[])
        self.nc.sync.drain()
    self.nc.all_engine_barrier()
    popped = self.nc._tile_sem_poison_stack.pop()
    assert popped is self._sem_poison
    self.nc.clear_and_free_semaphores(list(self.sems.allocated().values()))
    self.nc.all_engine_barrier()


_orig_commit = _TC._commit_instruction


def _patched_commit(self, inst, lazy_reg_writes=True):
    si = inst.sync_info
    if si is not None and si.on_wait and len(si.on_wait) > _MAX_WAITS and \
            inst.engine != _mb.EngineType.Unassigned:
        waits = list(si.on_wait)
        si.on_wait = waits[:_MAX_WAITS]
        eng = self.nc.engines[inst.engine]
        for w in waits[_MAX_WAITS:]:
            n = eng.nop()
            n.ins.sync_info = _mb.SyncInfo(on_wait=[w], on_update=[])
    return _orig_commit(self, inst, lazy_reg_writes)


if getattr(_TC, "_gat_patched", False) is False:
    _TC._drain_and_barrier = _patched_drain_and_barrier
    _TC._commit_instruction = _patched_commit
    _TC._gat_patched = True
# ---- end walrus compat ----


f32 = mybir.dt.float32
f16 = mybir.dt.float16
i16 = mybir.dt.int16
AF = mybir.ActivationFunctionType
ALU = mybir.AluOpType

P = 128
RANGE = 32768
NEG = 0.2
BN_EPS = 1e-5
GATHER_MAX = 896


def build_schedule(src_g, dst_g, n_nodes, n_cores):
    shard = n_nodes // n_cores
    n_tiles = (shard + P - 1) // P
    n_ranges = (n_nodes + RANGE - 1) // RANGE

    order = np.argsort(dst_g, kind="stable")
    src_s = src_g[order]
    dst_s = dst_g[order]
    core_bounds = np.searchsorted(dst_s, np.arange(n_cores + 1) * shard)

    counts = np.zeros((n_cores, n_tiles, n_ranges), np.int64)
    segs = [[[None] * n_ranges for _ in range(n_tiles)] for _ in range(n_cores)]
    for c in range(n_cores):
        lo, hi = core_bounds[c], core_bounds[c + 1]
        s = (src_s[lo:hi] - c * shard) % n_nodes
        d = dst_s[lo:hi] - c * shard
        t_id = d // P
        r_id = s // RANGE
        o2 = np.lexsort((s, r_id, t_id))
        s, d, t_id, r_id = s[o2], d[o2], t_id[o2], r_id[o2]
        key = t_id * n_ranges + r_id
        cnt = np.bincount(key, minlength=n_tiles * n_ranges).reshape(
            n_tiles, n_ranges)
        counts[c] = cnt
        offs = np.concatenate([[0], np.cumsum(cnt.ravel())])
        for t in range(n_tiles):
            for r in range(n_ranges):
                k = t * n_ranges + r
                sl = slice(offs[k], offs[k + 1])
                segs[c][t][r] = (s[sl], d[sl] - t * P)

    caps = counts.max(axis=0)
    caps = ((caps + P - 1) // P) * P
    cap_max = int(caps.max())
    assert cap_max <= 2048, cap_max

    G = 4
    n_grp2 = (n_tiles + G - 1) // G
    groups = [list(range(g * G, min((g + 1) * G, n_tiles)))
              for g in range(n_grp2)]
    o16 = np.zeros((n_tiles, n_ranges), np.int64)
    odl = np.zeros((n_tiles, n_ranges), np.int64)
    goff = np.zeros((n_tiles, n_ranges), np.int64)   # chunk offset in group buf
    gcap = np.zeros((n_grp2, n_ranges), np.int64)    # idx count per (group, r)
    go16 = np.zeros((n_grp2, n_ranges), np.int64)    # idx16 col of group block
    acc16 = accdl = 0
    for gi, ts in enumerate(groups):
        for r in range(n_ranges):
            go16[gi, r] = acc16
            coff = 0
            for t in ts:
                o16[t, r] = acc16
                goff[t, r] = coff
                acc16 += int(caps[t, r]) // 16
                coff += int(caps[t, r]) // P
            gcap[gi, r] = coff * P
    for t in range(n_tiles):
        for r in range(n_ranges):
            odl[t, r] = accdl
            accdl += int(caps[t, r]) // P
    idxcols, dlcols = acc16, accdl

    per_core = []
    for c in range(n_cores):
        idx_arr = np.zeros((16, idxcols), np.int16)
        dl_arr = np.full((P, dlcols), -1.0, np.float16)
        for t in range(n_tiles):
            for r in range(n_ranges):
                cap = int(caps[t, r])
                if cap == 0:
                    continue
                sseg, dseg = segs[c][t][r]
                n = len(sseg)
                i16v = np.zeros(cap, np.int16)
                i16v[:n] = (sseg - r * RANGE).astype(np.int16)
                dlv = np.full(cap, -1.0, np.float16)
                dlv[:n] = dseg.astype(np.float16)
                idx_arr[:, int(o16[t, r]): int(o16[t, r]) + cap // 16] = \
                    i16v.reshape(cap // 16, 16).T
                dl_arr[:, int(odl[t, r]): int(odl[t, r]) + cap // P] = \
                    dlv.reshape(cap // P, P).T
        per_core.append({
            "g_idx": np.tile(idx_arr, (8, 1)),
            "g_dl": dl_arr,
        })
    sched = {"caps": caps, "o16": o16, "odl": odl, "idxcols": idxcols,
             "dlcols": dlcols, "n_tiles": n_tiles, "n_ranges": n_ranges,
             "shard": shard, "cap_max": cap_max, "n_nodes": n_nodes,
             "groups": groups, "goff": goff, "gcap": gcap, "go16": go16}
    return sched, per_core


def build_program(sched):
    n_nodes = sched["n_nodes"]
    shard = sched["shard"]
    n_tiles, n_ranges = sched["n_tiles"], sched["n_ranges"]
    caps, o16, odl = sched["caps"], sched["o16"], sched["odl"]
    cap_max = sched["cap_max"]
    out_ch = P

    nc = bacc.Bacc("TRN2", target_bir_lowering=False, num_devices=8,
                   num_swdge_queues=4)
    x_in = nc.dram_tensor("x", [n_nodes, P], f16, kind="ExternalInput")
    w_in = nc.dram_tensor("w", [P, P], f32, kind="ExternalInput")
    asrc_in = nc.dram_tensor("att_src", [4, 32], f32, kind="ExternalInput")
    adst_in = nc.dram_tensor("att_dst", [4, 32], f32, kind="ExternalInput")
    gamma_in = nc.dram_tensor("gamma", [1, out_ch], f32, kind="ExternalInput")
    beta_in = nc.dram_tensor("beta", [1, out_ch], f32, kind="ExternalInput")
    gidx_in = nc.dram_tensor("g_idx", [P, sched["idxcols"]], i16,
                             kind="ExternalInput")
    gdl_in = nc.dram_tensor("g_dl", [P, sched["dlcols"]], f16,
                            kind="ExternalInput")
    iota_in = nc.dram_tensor("iota", [P, cap_max], f16, kind="ExternalInput")
    ident_in = nc.dram_tensor("ident", [P, P], f32, kind="ExternalInput")
    out_fin = nc.dram_tensor("out", [shard, out_ch], f32, kind="ExternalOutput")

    with tile.TileContext(nc) as tc:
        with tc.tile_pool(name="dram", bufs=1, space="DRAM") as drp, \
             tc.tile_pool(name="const", bufs=1) as cs:
            table = drp.tile([n_nodes, P], f32)
            bn_in = drp.tile([1, 2 * out_ch], f32)
            bn_out = drp.tile([1, 2 * out_ch], f32)

            # ---------------- constants ----------------
            identf = cs.tile([P, P], f32)
            nc.sync.dma_start(out=identf[:], in_=ident_in[:])
            identh = cs.tile([P, P], f16)
            nc.vector.tensor_copy(out=identh[:], in_=identf[:])
            iota_sb = cs.tile([P, cap_max], f16)
            nc.sync.dma_start(out=iota_sb[:], in_=iota_in[:])
            gidx_sb = cs.tile([P, sched["idxcols"]], i16)
            nc.sync.dma_start(out=gidx_sb[:], in_=gidx_in[:])
            gdl_sb = cs.tile([P, sched["dlcols"]], f16)
            nc.sync.dma_start(out=gdl_sb[:], in_=gdl_in[:])
            alpha_c = cs.tile([P, 1], f32)
            nc.vector.memset(alpha_c[:], NEG)
            ones_h = cs.tile([P, 1], f16)
            nc.vector.memset(ones_h[:], 1.0)
            ones1h = cs.tile([1, P], f16)
            nc.vector.memset(ones1h[:], 1.0)
            eps_c = cs.tile([1, 1], f32)
            nc.vector.memset(eps_c[:], BN_EPS)
            o_all = cs.tile([P, n_tiles, P], f16)

            # ---------------- phase 0: rhs_big = [W | W@A] fp16 ----------------
            rhs_big = cs.tile([P, P + 8], f16)
            with tc.tile_pool(name="p0", bufs=1) as p0, \
                 tc.tile_pool(name="p0ps", bufs=2, space="PSUM") as p0ps:
                wf = p0.tile([P, P], f32)
                nc.sync.dma_start(out=wf[:], in_=w_in[:])
                wh = p0.tile([P, P], f16)
                nc.vector.tensor_copy(out=wh[:], in_=wf[:])
                A_sb = p0.tile([P, 8], f32)
                nc.vector.memset(A_sb[:], 0.0)
                for hd in range(4):
                    nc.sync.dma_start(
                        out=A_sb[hd * 32:(hd + 1) * 32, hd:hd + 1],
                        in_=asrc_in[hd:hd + 1, :].rearrange("a b -> b a"))
                    nc.sync.dma_start(
                        out=A_sb[hd * 32:(hd + 1) * 32, 4 + hd:5 + hd],
                        in_=adst_in[hd:hd + 1, :].rearrange("a b -> b a"))
                Ah = p0.tile([P, 8], f16)
                nc.vector.tensor_copy(out=Ah[:], in_=A_sb[:])
                WT_ps = p0ps.tile([P, P], f16)
                nc.tensor.transpose(out=WT_ps[:], in_=wh[:], identity=identh[:])
                WT = p0.tile([P, P], f16)
                nc.scalar.copy(out=WT[:], in_=WT_ps[:])
                WA_ps = p0ps.tile([P, 8], f32)
                nc.tensor.matmul(out=WA_ps[:], lhsT=WT[:], rhs=Ah[:],
                                 start=True, stop=True)
                nc.vector.tensor_copy(out=rhs_big[:, 0:P], in_=wh[:])
                nc.vector.tensor_copy(out=rhs_big[:, P:P + 8], in_=WA_ps[:])

            # ---------------- phase 1: table build ----------------
            n_grp = (n_nodes + 511) // 512
            with tc.tile_pool(name="p1", bufs=4) as p1, \
                 tc.tile_pool(name="p1t", bufs=2, space="PSUM") as p1t, \
                 tc.tile_pool(name="p1h", bufs=2, space="PSUM") as p1h:
                for g in range(n_grp):
                    r0 = g * 512
                    rows = min(512, n_nodes - r0)
                    ptn = rows // 4
                    xq = p1.tile([P, 4, P], f16, tag="xq")
                    nc.sync.dma_start(
                        out=xq[:ptn],
                        in_=x_in[r0:r0 + rows].rearrange("(p j) c -> p j c",
                                                         j=4))
                    stg = p1.tile([P, 4, 74], f32, tag="stg")
                    ones_v = stg[:, :, 0:66].bitcast(f16).rearrange(
                        "p j (h c) -> p j h c", h=4)[:, :, :, 32:33]
                    nc.vector.memset(ones_v, 1.0)
                    for j in range(4):
                        xT_ps = p1t.tile([P, P], f16, tag="xt")
                        nc.tensor.transpose(out=xT_ps[:, :ptn],
                                            in_=xq[:ptn, j, :],
                                            identity=identh[:ptn, :ptn])
                        xT = p1.tile([P, P], f16, tag=f"xts{j % 2}")
                        nc.scalar.copy(out=xT[:, :ptn], in_=xT_ps[:, :ptn])
                        h_ps = p1h.tile([P, P + 8], f32, tag="hp")
                        nc.tensor.matmul(out=h_ps[:ptn], lhsT=xT[:, :ptn],
                                         rhs=rhs_big[:], start=True, stop=True)
                        nc.vector.tensor_copy(
                            out=stg[:ptn, j, 0:66].bitcast(f16).rearrange(
                                "p (h c) -> p h c", h=4)[:, :, 0:32],
                            in_=h_ps[:ptn, 0:P].rearrange(
                                "p (h c) -> p h c", h=4))
                        nc.scalar.copy(out=stg[:ptn, j, 66:74],
                                       in_=h_ps[:ptn, P:P + 8])
                    nc.sync.dma_start(
                        out=table[r0:r0 + rows].rearrange(
                            "(p j) c -> p j c", j=4)[:ptn, :, 0:74],
                        in_=stg[:ptn])

            # ---------------- phase 2: edges ----------------
            gq = [0]
            groups = sched["groups"]
            goff, gcap, go16 = sched["goff"], sched["gcap"], sched["go16"]
            with tc.tile_pool(name="gb", bufs=2) as gbp, \
                 tc.tile_pool(name="wk", bufs=5) as wk, \
                 tc.tile_pool(name="tl", bufs=3) as tlp, \
                 tc.tile_pool(name="agg", bufs=2, space="PSUM") as aggp, \
                 tc.tile_pool(name="aep", bufs=2, space="PSUM") as aep, \
                 tc.tile_pool(name="trp", bufs=2, space="PSUM") as trp, \
                 tc.tile_pool(name="stp", bufs=1, space="PSUM") as stp:
                stats_ps = stp.tile([1, 2 * out_ch], f32)
                for gi, ts in enumerate(groups):
                    # one gather per (group, range)
                    gtiles = {}
                    for r in range(n_ranges):
                        gc = int(gcap[gi, r])
                        if gc == 0:
                            continue
                        base = r * RANGE
                        end = min(base + RANGE, n_nodes)
                        gg = gbp.tile([P, gc // P, P], f32, tag=f"g{r}",
                                      name=f"g_{gi}_{r}")
                        off = 0
                        while off < gc:
                            piece = min(GATHER_MAX, gc - off)
                            nc.gpsimd.dma_gather(
                                out_ap=gg[:, off // P:(off + piece) // P, :],
                                in_ap=table[base:end, :],
                                idxs_ap=gidx_sb[:, int(go16[gi, r]) + off // 16:
                                                int(go16[gi, r]) + (off + piece) // 16],
                                num_idxs=piece,
                                num_idxs_reg=piece,
                                elem_size=P,
                                queue_num=gq[0] % 4,
                            )
                            gq[0] += 1
                            off += piece
                        gtiles[r] = gg
                    for t in ts:
                        d0 = t * P
                        pt = min(P, shard - d0)
                        adf = tlp.tile([P, 4], f32, tag="adf")
                        nc.sync.dma_start(out=adf[:pt],
                                          in_=table[d0:d0 + pt, 70:74])
                        adh = tlp.tile([P, 4], f16, tag="adh")
                        nc.vector.memset(adh[:], 0.0)
                        nc.vector.tensor_copy(out=adh[:pt], in_=adf[:pt])

                        psum_t = aggp.tile([P, out_ch + 4], f32, tag="agg",
                                           name=f"agg_{t}")
                        n_chunks = sum(int(caps[t, r]) // P
                                       for r in range(n_ranges))
                        done = 0
                        for r in range(n_ranges):
                            cap = int(caps[t, r])
                            if cap == 0:
                                continue
                            ch = cap // P
                            c0 = int(goff[t, r])
                            g = gtiles[r][:, c0:c0 + ch, :]
                            # Mc one-hot [e, d] batched over chunks
                            Mc = wk.tile([P, ch, P], f16, tag="mc",
                                         name=f"c_{t}_{r}")
                            dls = gdl_sb[:, int(odl[t, r]):int(odl[t, r]) + ch]
                            dl3 = bass.AP(tensor=dls.tensor, offset=dls.offset,
                                          ap=[dls.ap[0], dls.ap[1], [0, P]])
                            io = iota_sb[:, 0:P]
                            io3 = bass.AP(tensor=io.tensor, offset=io.offset,
                                          ap=[io.ap[0], [0, ch], io.ap[1]])
                            nc.vector.tensor_tensor(out=Mc[:], in0=dl3,
                                                    in1=io3, op=ALU.is_equal)
                            # McT chunks via PE transpose (2 per copy)
                            ae_ps = aep.tile([P, ch, 4], f32, tag="ae",
                                             name=f"ae_{t}_{r}")
                            for k2 in range(0, ch, 2):
                                kn = min(2, ch - k2)
                                mt_ps = trp.tile([P, 2 * P], f16, tag="tr")
                                for kk in range(kn):
                                    nc.tensor.transpose(
                                        out=mt_ps[:, kk * P:(kk + 1) * P],
                                        in_=Mc[:, k2 + kk, :],
                                        identity=identh[:])
                                mct = wk.tile([P, 2 * P], f16, tag="mcts")
                                nc.scalar.copy(out=mct[:, 0:kn * P],
                                               in_=mt_ps[:, 0:kn * P])
                                for kk in range(kn):
                                    nc.tensor.matmul(
                                        out=ae_ps[:, k2 + kk, :],
                                        lhsT=mct[:, kk * P:(kk + 1) * P],
                                        rhs=adh[:], start=True, stop=True)
                            # scores -> exp (leaky on V, exp on scalar)
                            s3 = wk.tile([P, ch, 4], f32, tag="s3",
                                         name=f"s_{t}_{r}")
                            nc.vector.tensor_tensor(out=s3[:],
                                                    in0=g[:, :, 66:70],
                                                    in1=ae_ps[:], op=ALU.add)
                            lr3 = wk.tile([P, ch, 4], f32, tag="lr",
                                          name=f"l_{t}_{r}")
                            nc.vector.scalar_tensor_tensor(
                                out=lr3[:], in0=s3[:], scalar=NEG,
                                in1=s3[:], op0=ALU.mult, op1=ALU.max)
                            ex3 = wk.tile([P, ch, 4], f16, tag="ex",
                                          name=f"e_{t}_{r}")
                            nc.scalar.activation(out=ex3[:], in_=lr3[:],
                                                 func=AF.Exp)
                            # me = [h*ex | ex] via ones-interleaved rows
                            me = wk.tile([P, ch, out_ch + 4], f16, tag="me",
                                         name=f"m_{t}_{r}")
                            hv = g[:, :, 0:66].bitcast(f16)
                            ex_b = bass.AP(
                                tensor=ex3.tensor, offset=ex3[:].offset,
                                ap=[ex3[:].ap[0], ex3[:].ap[1], ex3[:].ap[2],
                                    [0, 33]])
                            nc.vector.tensor_tensor(
                                out=me[:].rearrange(
                                    "p a (h c) -> p a h c", h=4),
                                in0=hv.rearrange("p a (h c) -> p a h c", h=4),
                                in1=ex_b, op=ALU.mult)
                            for k in range(ch):
                                nc.tensor.matmul(out=psum_t[:],
                                                 lhsT=Mc[:, k, :],
                                                 rhs=me[:, k, :],
                                                 start=(done == 0),
                                                 stop=(done == n_chunks - 1))
                                done += 1

                        # ---- per-tile epilogue ----
                        pv = psum_t[:pt].rearrange("p (h c) -> p h c", h=4)
                        den = tlp.tile([P, 4], f32, tag="den")
                        nc.vector.tensor_scalar(out=den[:pt],
                                                in0=pv[:, :, 32:33],
                                                scalar1=1e-16, scalar2=None,
                                                op0=ALU.add)
                        rden = tlp.tile([P, 4], f32, tag="rden")
                        nc.vector.reciprocal(out=rden[:pt], in_=den[:pt])
                        rv = rden[:pt]
                        rd_b = bass.AP(tensor=rv.tensor, offset=rv.offset,
                                       ap=[rv.ap[0], rv.ap[1], [0, 32]])
                        nc.vector.tensor_tensor(
                            out=o_all[:pt, t, :].rearrange("p (h c) -> p h c", h=4),
                            in0=pv[:, :, 0:32],
                            in1=rd_b, op=ALU.mult)
                        sq = tlp.tile([P, P], f16, tag="sq")
                        nc.vector.tensor_tensor(out=sq[:pt], in0=o_all[:pt, t, :],
                                                in1=o_all[:pt, t, :], op=ALU.mult)
                        nc.tensor.matmul(out=stats_ps[0:1, 0:P],
                                         lhsT=ones_h[:pt], rhs=o_all[:pt, t, :],
                                         start=(t == 0), stop=(t == n_tiles - 1))
                        nc.tensor.matmul(out=stats_ps[0:1, P:2 * P],
                                         lhsT=ones_h[:pt], rhs=sq[:pt],
                                         start=(t == 0), stop=(t == n_tiles - 1))

                # ---------------- BN stats allreduce ----------------
                with tc.tile_pool(name="bn", bufs=1) as bnp, \
                     tc.tile_pool(name="bnps", bufs=1, space="PSUM") as bnps:
                    stats_sb = bnp.tile([1, 2 * out_ch], f32)
                    nc.scalar.copy(out=stats_sb[:], in_=stats_ps[:])
                    nc.sync.dma_start(out=bn_in[:], in_=stats_sb[:])
                    nc.gpsimd.collective_compute(
                        "AllReduce", ALU.add,
                        replica_groups=[list(range(8))],
                        ins=[bn_in[:]], outs=[bn_out[:]],
                    )
                    tots = bnp.tile([1, 2 * out_ch], f32)
                    nc.sync.dma_start(out=tots[:], in_=bn_out[:])
                    gam = bnp.tile([1, out_ch], f32)
                    nc.sync.dma_start(out=gam[:], in_=gamma_in[:])
                    bet = bnp.tile([1, out_ch], f32)
                    nc.sync.dma_start(out=bet[:], in_=beta_in[:])
                    mu = bnp.tile([1, out_ch], f32)
                    nc.vector.tensor_scalar(out=mu[:], in0=tots[:, 0:out_ch],
                                            scalar1=1.0 / n_nodes,
                                            scalar2=None, op0=ALU.mult)
                    ex2 = bnp.tile([1, out_ch], f32)
                    nc.vector.tensor_scalar(out=ex2[:], in0=tots[:, out_ch:],
                                            scalar1=1.0 / n_nodes,
                                            scalar2=None, op0=ALU.mult)
                    musq = bnp.tile([1, out_ch], f32)
                    nc.vector.tensor_tensor(out=musq[:], in0=mu[:], in1=mu[:],
                                            op=ALU.mult)
                    var = bnp.tile([1, out_ch], f32)
                    nc.vector.tensor_tensor(out=var[:], in0=ex2[:],
                                            in1=musq[:], op=ALU.subtract)
                    sd = bnp.tile([1, out_ch], f32)
                    nc.scalar.activation(out=sd[:], in_=var[:], func=AF.Sqrt,
                                         bias=eps_c[0:1, 0:1], scale=1.0)
                    rs = bnp.tile([1, out_ch], f32)
                    nc.vector.reciprocal(out=rs[:], in_=sd[:])
                    scale_r = bnp.tile([1, out_ch], f32)
                    nc.vector.tensor_tensor(out=scale_r[:], in0=rs[:],
                                            in1=gam[:], op=ALU.mult)
                    mshift = bnp.tile([1, out_ch], f32)
                    nc.vector.tensor_tensor(out=mshift[:], in0=mu[:],
                                            in1=scale_r[:], op=ALU.mult)
                    shift_r = bnp.tile([1, out_ch], f32)
                    nc.vector.tensor_tensor(out=shift_r[:], in0=bet[:],
                                            in1=mshift[:], op=ALU.subtract)
                    scsh = bnp.tile([1, 2 * out_ch], f16)
                    nc.vector.tensor_copy(out=scsh[:, 0:out_ch],
                                          in_=scale_r[:])
                    nc.vector.tensor_copy(out=scsh[:, out_ch:], in_=shift_r[:])
                    bc_ps = bnps.tile([P, 2 * out_ch], f32)
                    nc.tensor.matmul(out=bc_ps[:], lhsT=ones1h[:],
                                     rhs=scsh[:], start=True, stop=True)
                    scale_bc = cs.tile([P, out_ch], f32)
                    nc.vector.tensor_copy(out=scale_bc[:],
                                          in_=bc_ps[:, 0:out_ch])
                    shift_bc = cs.tile([P, out_ch], f32)
                    nc.scalar.copy(out=shift_bc[:], in_=bc_ps[:, out_ch:])

                # ---------------- phase 3: normalize ----------------
                with tc.tile_pool(name="p3", bufs=3) as p3:
                    for t in range(n_tiles):
                        d0 = t * P
                        pt = min(P, shard - d0)
                        y_t = p3.tile([P, out_ch], f32, tag="y3")
                        nc.vector.tensor_tensor(out=y_t[:pt],
                                                in0=o_all[:pt, t, :],
                                                in1=scale_bc[:pt],
                                                op=ALU.mult)
                        nc.vector.tensor_tensor(out=y_t[:pt], in0=y_t[:pt],
                                                in1=shift_bc[:pt], op=ALU.add)
                        z_t = p3.tile([P, out_ch], f32, tag="z3")
                        nc.scalar.activation(out=z_t[:pt], in_=y_t[:pt],
                                             func=AF.Prelu,
                                             alpha=alpha_c[:pt, 0:1])
                        nc.sync.dma_start(out=out_fin[d0:d0 + pt, :],
                                          in_=z_t[:pt])

    nc.compile()
    return nc


def run(x, edge_index, W, att_src, att_dst, bias, gamma, beta,
        n_cores=8, trace=False, tmpdir=None):
    n_nodes, in_ch = x.shape
    shard = n_nodes // n_cores

    src = np.asarray(edge_index[0], np.int64)
    dst = np.asarray(edge_index[1], np.int64)
    sched, per_core = build_schedule(src, dst, n_nodes, n_cores)
    nc = build_program(sched)

    x = np.asarray(x, np.float32)
    iota_np = np.tile(np.arange(sched["cap_max"], dtype=np.float16), (P, 1))
    in_maps = []
    for c in range(n_cores):
        in_maps.append({
            "x": np.roll(x, -c * shard, axis=0).astype(np.float16),
            "w": np.asarray(W, np.float32),
            "att_src": np.asarray(att_src, np.float32),
            "att_dst": np.asarray(att_dst, np.float32),
            "gamma": np.asarray(gamma, np.float32).reshape(1, -1),
            "beta": np.asarray(beta, np.float32).reshape(1, -1),
            "g_idx": per_core[c]["g_idx"],
            "g_dl": per_core[c]["g_dl"],
            "iota": iota_np,
            "ident": np.eye(P, dtype=np.float32),
        })
    res = run_bass_kernel_spmd(nc, in_maps, core_ids=list(range(n_cores)),
                               trace=trace, tmpdir=tmpdir)
    out = np.concatenate([res.results[c]["out"] for c in range(n_cores)],
                         axis=0)
    return out, res


def _install_ntff_hook():
    """Best-effort NTFF profile hook for trace mode (missing in this image)."""
    import sys, types
    try:
        import antenv.axon_hooks  # noqa: F401
        return
    except ImportError:
        pass
    try:
        from trn_agent_boot.trn_boot import _ntff_profile_via_ctypes
        hook = _ntff_profile_via_ctypes('/opt/axon/libaxon_pjrt.so')
    except Exception:
        hook = None
    mod = types.ModuleType("antenv.axon_hooks")
    mod.get_axon_ntff_profile_hook = lambda: hook
    mod.set_axon_ntff_profile_hook = lambda h: None
    sys.modules["antenv.axon_hooks"] = mod


def kernel(**inputs):
    x = np.asarray(inputs["x"], np.float32)
    edge_index = np.asarray(inputs["edge_index"])
    W = np.asarray(inputs["W"], np.float32)
    att_src = np.asarray(inputs["att_src"], np.float32)
    att_dst = np.asarray(inputs["att_dst"], np.float32)
    bias = np.asarray(inputs["bias"], np.float32)
    gamma = np.asarray(inputs["gamma"], np.float32)
    beta = np.asarray(inputs["beta"], np.float32)
    import os
    trace = bool(os.environ.get("GAT_TRACE"))
    if trace:
        _install_ntff_hook()
    tmpdir = os.environ.get("GAT_TRACE_DIR")
    out, res = run(x, edge_index, W, att_src, att_dst, bias, gamma, beta,
                   trace=trace, tmpdir=tmpdir)
    kernel.last_result = res
    return out


# revision 14
# speedup vs baseline: 1.2978x; 1.0617x over previous
"""GATConvBlock (GAT attention + BatchNorm + LeakyReLU) on 8 Trainium2
NeuronCores. Self-contained: host-side edge scheduling + Bass/Tile program +
SPMD execution via concourse.

Strategy: dst-sharded graph parallelism, fp16 datapath. Each core owns 12500
dst nodes; x is rotated per core so its shard is table rows 0:12500. Phase 1
computes a full per-node table row [h fp16 x128 | a_src f32 x4 | a_dst f32 x4]
(512B stride) with interleaved 4-tile loads. Phase 2 processes edges grouped
by (dst-tile, src-range): int16 dma_gather of source rows, membership matrices
built as batched vector ops (interval masks from host-precomputed start/end
columns; one-hot via batched is_equal), per-edge a_dst via small PE matmuls,
segment softmax + aggregation via membership matmuls accumulating in PSUM.
BatchNorm stats via ones-matmuls into one PSUM tile + AllReduce."""
import numpy as np

import concourse.bass as bass
import concourse.bacc as bacc
import concourse.tile as tile
from concourse import mybir
from concourse.bass_utils import run_bass_kernel_spmd

# ---- walrus compat: split multi-wait sync_info (this toolchain rejects >1) ----
from concourse import mybir as _mb
from concourse.tile import TileContext as _TC
from concourse.vector_clock import ScopedClock as _SC

_MAX_WAITS = 1


def _patched_drain_and_barrier(self, tick_clock, wait_clock):
    drain_inst = self.nc.sync.drain()
    wait_clock.add_sem_waits(drain_inst.ins, _SC({None: tick_clock.global_clock}))
    si = drain_inst.ins.sync_info
    waits = list(si.on_wait or [])
    if len(waits) > _MAX_WAITS:
        si.on_wait = waits[:_MAX_WAITS]
        for w in waits[_MAX_WAITS:]:
            n = self.nc.sync.nop()
            n.ins.sync_info = _mb.SyncInfo(on_wait=[w], on_update=[])
        self.nc.sync.drain()
    self.nc.all_engine_barrier()
    popped = self.nc._tile_sem_poison_stack.pop()
    assert popped is self._sem_poison
    self.nc.clear_and_free_semaphores(list(self.sems.allocated().values()))
    self.nc.all_engine_barrier()


_orig_commit = _TC._commit_instruction


def _patched_commit(self, inst, lazy_reg_writes=True):
    si = inst.sync_info
    if si is not None and si.on_wait and len(si.on_wait) > _MAX_WAITS and \
            inst.engine != _mb.EngineType.Unassigned:
        waits = list(si.on_wait)
        si.on_wait = waits[:_MAX_WAITS]
        eng = self.nc.engines[inst.engine]
        for w in waits[_MAX_WAITS:]:
            n = eng.nop()
            n.ins.sync_info = _mb.SyncInfo(on_wait=[w], on_update=[])
    return _orig_commit(self, inst, lazy_reg_writes)


if getattr(_TC, "_gat_patched", False) is False:
    _TC._drain_and_barrier = _patched_drain_and_barrier
    _TC._commit_instruction = _patched_commit
    _TC._gat_patched = True
# ---- end walrus compat ----


f32 = mybir.dt.float32
f16 = mybir.dt.float16
i16 = mybir.dt.int16
AF = mybir.ActivationFunctionType
ALU = mybir.AluOpType

P = 128
RANGE = 32768
NEG = 0.2
BN_EPS = 1e-5
GATHER_MAX = 896


def build_schedule(src_g, dst_g, n_nodes, n_cores):
    shard = n_nodes // n_cores
    n_tiles = (shard + P - 1) // P
    n_ranges = (n_nodes + RANGE - 1) // RANGE

    order = np.argsort(dst_g, kind="stable")
    src_s = src_g[order]
    dst_s = dst_g[order]
    core_bounds = np.searchsorted(dst_s, np.arange(n_cores + 1) * shard)

    counts = np.zeros((n_cores, n_tiles, n_ranges), np.int64)
    segs = [[[None] * n_ranges for _ in range(n_tiles)] for _ in range(n_cores)]
    for c in range(n_cores):
        lo, hi = core_bounds[c], core_bounds[c + 1]
        s = (src_s[lo:hi] - c * shard) % n_nodes
        d = dst_s[lo:hi] - c * shard
        t_id = d // P
        r_id = s // RANGE
        o2 = np.lexsort((s, r_id, t_id))
        s, d, t_id, r_id = s[o2], d[o2], t_id[o2], r_id[o2]
        key = t_id * n_ranges + r_id
        cnt = np.bincount(key, minlength=n_tiles * n_ranges).reshape(
            n_tiles, n_ranges)
        counts[c] = cnt
        offs = np.concatenate([[0], np.cumsum(cnt.ravel())])
        for t in range(n_tiles):
            for r in range(n_ranges):
                k = t * n_ranges + r
                sl = slice(offs[k], offs[k + 1])
                segs[c][t][r] = (s[sl], d[sl] - t * P)

    caps = counts.max(axis=0)
    caps = ((caps + P - 1) // P) * P
    cap_max = int(caps.max())
    assert cap_max <= 2048, cap_max

    G = 4
    n_grp2 = (n_tiles + G - 1) // G
    groups = [list(range(g * G, min((g + 1) * G, n_tiles)))
              for g in range(n_grp2)]
    o16 = np.zeros((n_tiles, n_ranges), np.int64)
    odl = np.zeros((n_tiles, n_ranges), np.int64)
    goff = np.zeros((n_tiles, n_ranges), np.int64)   # chunk offset in group buf
    gcap = np.zeros((n_grp2, n_ranges), np.int64)    # idx count per (group, r)
    go16 = np.zeros((n_grp2, n_ranges), np.int64)    # idx16 col of group block
    acc16 = accdl = 0
    for gi, ts in enumerate(groups):
        for r in range(n_ranges):
            go16[gi, r] = acc16
            coff = 0
            for t in ts:
                o16[t, r] = acc16
                goff[t, r] = coff
                acc16 += int(caps[t, r]) // 16
                coff += int(caps[t, r]) // P
            gcap[gi, r] = coff * P
    gdl = np.zeros((n_grp2, n_ranges), np.int64)   # dl col of group block
    for gi, ts in enumerate(groups):
        for r in range(n_ranges):
            gdl[gi, r] = accdl
            for t in ts:
                odl[t, r] = accdl
                accdl += int(caps[t, r]) // P
    idxcols, dlcols = acc16, accdl

    per_core = []
    for c in range(n_cores):
        idx_arr = np.zeros((16, idxcols), np.int16)
        dl_arr = np.full((P, dlcols), -1.0, np.float16)
        for t in range(n_tiles):
            for r in range(n_ranges):
                cap = int(caps[t, r])
                if cap == 0:
                    continue
                sseg, dseg = segs[c][t][r]
                n = len(sseg)
                i16v = np.zeros(cap, np.int16)
                i16v[:n] = (sseg - r * RANGE).astype(np.int16)
                dlv = np.full(cap, -1.0, np.float16)
                dlv[:n] = dseg.astype(np.float16)
                idx_arr[:, int(o16[t, r]): int(o16[t, r]) + cap // 16] = \
                    i16v.reshape(cap // 16, 16).T
                dl_arr[:, int(odl[t, r]): int(odl[t, r]) + cap // P] = \
                    dlv.reshape(cap // P, P).T
        per_core.append({
            "g_idx": np.tile(idx_arr, (8, 1)),
            "g_dl": dl_arr,
        })
    sched = {"caps": caps, "o16": o16, "odl": odl, "idxcols": idxcols,
             "dlcols": dlcols, "n_tiles": n_tiles, "n_ranges": n_ranges,
             "shard": shard, "cap_max": cap_max, "n_nodes": n_nodes,
             "groups": groups, "goff": goff, "gcap": gcap, "go16": go16,
             "gdl": gdl}
    return sched, per_core


def build_program(sched):
    n_nodes = sched["n_nodes"]
    shard = sched["shard"]
    n_tiles, n_ranges = sched["n_tiles"], sched["n_ranges"]
    caps, o16, odl = sched["caps"], sched["o16"], sched["odl"]
    cap_max = sched["cap_max"]
    out_ch = P

    nc = bacc.Bacc("TRN2", target_bir_lowering=False, num_devices=8,
                   num_swdge_queues=4)
    x_in = nc.dram_tensor("x", [n_nodes, P], f16, kind="ExternalInput")
    w_in = nc.dram_tensor("w", [P, P], f32, kind="ExternalInput")
    asrc_in = nc.dram_tensor("att_src", [4, 32], f32, kind="ExternalInput")
    adst_in = nc.dram_tensor("att_dst", [4, 32], f32, kind="ExternalInput")
    gamma_in = nc.dram_tensor("gamma", [1, out_ch], f32, kind="ExternalInput")
    beta_in = nc.dram_tensor("beta", [1, out_ch], f32, kind="ExternalInput")
    gidx_in = nc.dram_tensor("g_idx", [P, sched["idxcols"]], i16,
                             kind="ExternalInput")
    gdl_in = nc.dram_tensor("g_dl", [P, sched["dlcols"]], f16,
                            kind="ExternalInput")
    iota_in = nc.dram_tensor("iota", [P, cap_max], f16, kind="ExternalInput")
    ident_in = nc.dram_tensor("ident", [P, P], f32, kind="ExternalInput")
    out_fin = nc.dram_tensor("out", [shard, out_ch], f32, kind="ExternalOutput")

    with tile.TileContext(nc) as tc:
        with tc.tile_pool(name="dram", bufs=1, space="DRAM") as drp, \
             tc.tile_pool(name="const", bufs=1) as cs:
            table = drp.tile([n_nodes, P], f32)
            bn_in = drp.tile([1, 2 * out_ch], f32)
            bn_out = drp.tile([1, 2 * out_ch], f32)

            # ---------------- constants ----------------
            identf = cs.tile([P, P], f32)
            nc.sync.dma_start(out=identf[:], in_=ident_in[:])
            identh = cs.tile([P, P], f16)
            nc.vector.tensor_copy(out=identh[:], in_=identf[:])
            iota_sb = cs.tile([P, cap_max], f16)
            nc.sync.dma_start(out=iota_sb[:], in_=iota_in[:])
            gidx_sb = cs.tile([P, sched["idxcols"]], i16)
            nc.sync.dma_start(out=gidx_sb[:], in_=gidx_in[:])
            gdl_sb = cs.tile([P, sched["dlcols"]], f16)
            nc.sync.dma_start(out=gdl_sb[:], in_=gdl_in[:])
            alpha_c = cs.tile([P, 1], f32)
            nc.vector.memset(alpha_c[:], NEG)
            ones_h = cs.tile([P, 1], f16)
            nc.vector.memset(ones_h[:], 1.0)
            ones1h = cs.tile([1, P], f16)
            nc.vector.memset(ones1h[:], 1.0)
            eps_c = cs.tile([1, 1], f32)
            nc.vector.memset(eps_c[:], BN_EPS)
            o_all = cs.tile([P, n_tiles, P], f16)

            # ---------------- phase 0: rhs_big = [W | W@A] fp16 ----------------
            rhs_big = cs.tile([P, P + 8], f16)
            with tc.tile_pool(name="p0", bufs=1) as p0, \
                 tc.tile_pool(name="p0ps", bufs=2, space="PSUM") as p0ps:
                wf = p0.tile([P, P], f32)
                nc.sync.dma_start(out=wf[:], in_=w_in[:])
                wh = p0.tile([P, P], f16)
                nc.vector.tensor_copy(out=wh[:], in_=wf[:])
                A_sb = p0.tile([P, 8], f32)
                nc.vector.memset(A_sb[:], 0.0)
                for hd in range(4):
                    nc.sync.dma_start(
                        out=A_sb[hd * 32:(hd + 1) * 32, hd:hd + 1],
                        in_=asrc_in[hd:hd + 1, :].rearrange("a b -> b a"))
                    nc.sync.dma_start(
                        out=A_sb[hd * 32:(hd + 1) * 32, 4 + hd:5 + hd],
                        in_=adst_in[hd:hd + 1, :].rearrange("a b -> b a"))
                Ah = p0.tile([P, 8], f16)
                nc.vector.tensor_copy(out=Ah[:], in_=A_sb[:])
                WT_ps = p0ps.tile([P, P], f16)
                nc.tensor.transpose(out=WT_ps[:], in_=wh[:], identity=identh[:])
                WT = p0.tile([P, P], f16)
                nc.scalar.copy(out=WT[:], in_=WT_ps[:])
                WA_ps = p0ps.tile([P, 8], f32)
                nc.tensor.matmul(out=WA_ps[:], lhsT=WT[:], rhs=Ah[:],
                                 start=True, stop=True)
                nc.vector.tensor_copy(out=rhs_big[:, 0:P], in_=wh[:])
                nc.vector.tensor_copy(out=rhs_big[:, P:P + 8], in_=WA_ps[:])

            # ---------------- phase 1: table build ----------------
            n_grp = (n_nodes + 511) // 512
            with tc.tile_pool(name="p1", bufs=4) as p1, \
                 tc.tile_pool(name="p1t", bufs=2, space="PSUM") as p1t, \
                 tc.tile_pool(name="p1h", bufs=2, space="PSUM") as p1h:
                for g in range(n_grp):
                    r0 = g * 512
                    rows = min(512, n_nodes - r0)
                    ptn = rows // 4
                    xq = p1.tile([P, 4, P], f16, tag="xq")
                    nc.sync.dma_start(
                        out=xq[:ptn],
                        in_=x_in[r0:r0 + rows].rearrange("(p j) c -> p j c",
                                                         j=4))
                    stg = p1.tile([P, 4, 74], f32, tag="stg")
                    ones_v = stg[:, :, 0:66].bitcast(f16).rearrange(
                        "p j (h c) -> p j h c", h=4)[:, :, :, 32:33]
                    nc.vector.memset(ones_v, 1.0)
                    for j in range(4):
                        xT_ps = p1t.tile([P, P], f16, tag="xt")
                        nc.tensor.transpose(out=xT_ps[:, :ptn],
                                            in_=xq[:ptn, j, :],
                                            identity=identh[:ptn, :ptn])
                        xT = p1.tile([P, P], f16, tag=f"xts{j % 2}")
                        nc.scalar.copy(out=xT[:, :ptn], in_=xT_ps[:, :ptn])
                        h_ps = p1h.tile([P, P + 8], f32, tag="hp")
                        nc.tensor.matmul(out=h_ps[:ptn], lhsT=xT[:, :ptn],
                                         rhs=rhs_big[:], start=True, stop=True)
                        nc.vector.tensor_copy(
                            out=stg[:ptn, j, 0:66].bitcast(f16).rearrange(
                                "p (h c) -> p h c", h=4)[:, :, 0:32],
                            in_=h_ps[:ptn, 0:P].rearrange(
                                "p (h c) -> p h c", h=4))
                        nc.scalar.copy(out=stg[:ptn, j, 66:74],
                                       in_=h_ps[:ptn, P:P + 8])
                    nc.sync.dma_start(
                        out=table[r0:r0 + rows].rearrange(
                            "(p j) c -> p j c", j=4)[:ptn, :, 0:74],
                        in_=stg[:ptn])

            # ---------------- phase 2: edges ----------------
            gq = [0]
            groups = sched["groups"]
            goff, gcap, go16 = sched["goff"], sched["gcap"], sched["go16"]
            gdl = sched["gdl"]
            with tc.tile_pool(name="stp", bufs=1, space="PSUM") as stp:
              stats_ps = stp.tile([1, 2 * out_ch], f32)
              with tc.tile_pool(name="gb", bufs=2) as gbp, \
                 tc.tile_pool(name="wk", bufs=2) as wk, \
                 tc.tile_pool(name="tl", bufs=3) as tlp, \
                 tc.tile_pool(name="agg", bufs=4, space="PSUM") as aggp, \
                 tc.tile_pool(name="aep", bufs=1, space="PSUM") as aep, \
                 tc.tile_pool(name="trp", bufs=2, space="PSUM") as trp:
                for gi, ts in enumerate(groups):
                    # one gather per (group, range)
                    gtiles = {}
                    for r in range(n_ranges):
                        gc = int(gcap[gi, r])
                        if gc == 0:
                            continue
                        base = r * RANGE
                        end = min(base + RANGE, n_nodes)
                        gg = gbp.tile([P, gc // P, P], f32, tag=f"g{r}",
                                      name=f"g_{gi}_{r}")
                        off = 0
                        while off < gc:
                            piece = min(GATHER_MAX, gc - off)
                            nc.gpsimd.dma_gather(
                                out_ap=gg[:, off // P:(off + piece) // P, :],
                                in_ap=table[base:end, :],
                                idxs_ap=gidx_sb[:, int(go16[gi, r]) + off // 16:
                                                int(go16[gi, r]) + (off + piece) // 16],
                                num_idxs=piece,
                                num_idxs_reg=piece,
                                elem_size=P,
                                queue_num=gq[0] % 4,
                            )
                            gq[0] += 1
                            off += piece
                        gtiles[r] = gg
                    # per-tile dst attention + agg psum
                    adhs = {}
                    psums = {}
                    n_chunks = {}
                    done = {}
                    for t in ts:
                        d0 = t * P
                        pt = min(P, shard - d0)
                        adf = tlp.tile([P, 4], f32, tag=f"adf{t % 2}")
                        nc.sync.dma_start(out=adf[:pt],
                                          in_=table[d0:d0 + pt, 70:74])
                        adh = tlp.tile([P, 4], f16, tag=f"adh{t % 2}")
                        nc.vector.memset(adh[:], 0.0)
                        nc.vector.tensor_copy(out=adh[:pt], in_=adf[:pt])
                        adhs[t] = adh
                        psums[t] = aggp.tile([P, out_ch + 4], f32, tag="agg",
                                             name=f"agg_{t}")
                        n_chunks[t] = sum(int(caps[t, r]) // P
                                          for r in range(n_ranges))
                        done[t] = 0
                    for r in range(n_ranges):
                        gc = int(gcap[gi, r])
                        if gc == 0:
                            continue
                        gch = gc // P
                        gg = gtiles[r]
                        # chunk -> tile map for this (group, r)
                        ctile = []
                        for t in ts:
                            ctile += [t] * (int(caps[t, r]) // P)
                        # Mc one-hot [e, d] batched over all group chunks
                        Mc = wk.tile([P, gch, P], f16, tag="mc",
                                     name=f"c_{gi}_{r}")
                        dls = gdl_sb[:, int(gdl[gi, r]):int(gdl[gi, r]) + gch]
                        dl3 = bass.AP(tensor=dls.tensor, offset=dls.offset,
                                      ap=[dls.ap[0], dls.ap[1], [0, P]])
                        io = iota_sb[:, 0:P]
                        io3 = bass.AP(tensor=io.tensor, offset=io.offset,
                                      ap=[io.ap[0], [0, gch], io.ap[1]])
                        nc.vector.tensor_tensor(out=Mc[:], in0=dl3,
                                                in1=io3, op=ALU.is_equal)
                        # McT chunks via PE transpose (2 per copy); ae matmuls
                        ae_ps = aep.tile([P, gch, 4], f32, tag="ae",
                                         name=f"ae_{gi}_{r}")
                        for k2 in range(0, gch, 2):
                            kn = min(2, gch - k2)
                            mt_ps = trp.tile([P, 2 * P], f16, tag="tr")
                            for kk in range(kn):
                                nc.tensor.transpose(
                                    out=mt_ps[:, kk * P:(kk + 1) * P],
                                    in_=Mc[:, k2 + kk, :],
                                    identity=identh[:])
                            mct = wk.tile([P, 2 * P], f16, tag="mcts")
                            nc.scalar.copy(out=mct[:, 0:kn * P],
                                           in_=mt_ps[:, 0:kn * P])
                            for kk in range(kn):
                                nc.tensor.matmul(
                                    out=ae_ps[:, k2 + kk, :],
                                    lhsT=mct[:, kk * P:(kk + 1) * P],
                                    rhs=adhs[ctile[k2 + kk]][:],
                                    start=True, stop=True)
                        # scores -> exp (leaky on V, exp on scalar)
                        s3 = wk.tile([P, gch, 4], f32, tag="s3",
                                     name=f"s_{gi}_{r}")
                        nc.vector.tensor_tensor(out=s3[:],
                                                in0=gg[:, :, 66:70],
                                                in1=ae_ps[:], op=ALU.add)
                        lr3 = wk.tile([P, gch, 4], f32, tag="lr",
                                      name=f"l_{gi}_{r}")
                        nc.vector.scalar_tensor_tensor(
                            out=lr3[:], in0=s3[:], scalar=NEG,
                            in1=s3[:], op0=ALU.mult, op1=ALU.max)
                        ex3 = wk.tile([P, gch, 4], f16, tag="ex",
                                      name=f"e_{gi}_{r}")
                        nc.scalar.activation(out=ex3[:], in_=lr3[:],
                                             func=AF.Exp)
                        # me = [h*ex | ex] via ones-interleaved rows
                        me = wk.tile([P, gch, out_ch + 4], f16, tag="me",
                                     name=f"m_{gi}_{r}")
                        hv = gg[:, :, 0:66].bitcast(f16)
                        ex_b = bass.AP(
                            tensor=ex3.tensor, offset=ex3[:].offset,
                            ap=[ex3[:].ap[0], ex3[:].ap[1], ex3[:].ap[2],
                                [0, 33]])
                        nc.vector.tensor_tensor(
                            out=me[:].rearrange(
                                "p a (h c) -> p a h c", h=4),
                            in0=hv.rearrange("p a (h c) -> p a h c", h=4),
                            in1=ex_b, op=ALU.mult)
                        for k in range(gch):
                            t = ctile[k]
                            nc.tensor.matmul(out=psums[t][:],
                                             lhsT=Mc[:, k, :],
                                             rhs=me[:, k, :],
                                             start=(done[t] == 0),
                                             stop=(done[t] == n_chunks[t] - 1))
                            done[t] += 1
                    # ---- per-tile epilogues ----
                    for t in ts:
                        d0 = t * P
                        pt = min(P, shard - d0)
                        psum_t = psums[t]
                        pv = psum_t[:pt].rearrange("p (h c) -> p h c", h=4)
                        den = tlp.tile([P, 4], f32, tag="den")
                        nc.vector.tensor_scalar(out=den[:pt],
                                                in0=pv[:, :, 32:33],
                                                scalar1=1e-16, scalar2=None,
                                                op0=ALU.add)
                        rden = tlp.tile([P, 4], f32, tag="rden")
                        nc.vector.reciprocal(out=rden[:pt], in_=den[:pt])
                        rv = rden[:pt]
                        rd_b = bass.AP(tensor=rv.tensor, offset=rv.offset,
                                       ap=[rv.ap[0], rv.ap[1], [0, 32]])
                        nc.vector.tensor_tensor(
                            out=o_all[:pt, t, :].rearrange("p (h c) -> p h c", h=4),
                            in0=pv[:, :, 0:32],
                            in1=rd_b, op=ALU.mult)
                        sq = tlp.tile([P, P], f16, tag="sq")
                        nc.vector.tensor_tensor(out=sq[:pt], in0=o_all[:pt, t, :],
                                                in1=o_all[:pt, t, :], op=ALU.mult)
                        nc.tensor.matmul(out=stats_ps[0:1, 0:P],
                                         lhsT=ones_h[:pt], rhs=o_all[:pt, t, :],
                                         start=(t == 0), stop=(t == n_tiles - 1))
                        nc.tensor.matmul(out=stats_ps[0:1, P:2 * P],
                                         lhsT=ones_h[:pt], rhs=sq[:pt],
                                         start=(t == 0), stop=(t == n_tiles - 1))

                # ---------------- BN stats allreduce ----------------
              if True:
                with tc.tile_pool(name="bn", bufs=1) as bnp, \
                     tc.tile_pool(name="bnps", bufs=1, space="PSUM") as bnps:
                    stats_sb = bnp.tile([1, 2 * out_ch], f32)
                    nc.scalar.copy(out=stats_sb[:], in_=stats_ps[:])
                    nc.sync.dma_start(out=bn_in[:], in_=stats_sb[:])
                    nc.gpsimd.collective_compute(
                        "AllReduce", ALU.add,
                        replica_groups=[list(range(8))],
                        ins=[bn_in[:]], outs=[bn_out[:]],
                    )
                    tots = bnp.tile([1, 2 * out_ch], f32)
                    nc.sync.dma_start(out=tots[:], in_=bn_out[:])
                    gam = bnp.tile([1, out_ch], f32)
                    nc.sync.dma_start(out=gam[:], in_=gamma_in[:])
                    bet = bnp.tile([1, out_ch], f32)
                    nc.sync.dma_start(out=bet[:], in_=beta_in[:])
                    mu = bnp.tile([1, out_ch], f32)
                    nc.vector.tensor_scalar(out=mu[:], in0=tots[:, 0:out_ch],
                                            scalar1=1.0 / n_nodes,
                                            scalar2=None, op0=ALU.mult)
                    ex2 = bnp.tile([1, out_ch], f32)
                    nc.vector.tensor_scalar(out=ex2[:], in0=tots[:, out_ch:],
                                            scalar1=1.0 / n_nodes,
                                            scalar2=None, op0=ALU.mult)
                    musq = bnp.tile([1, out_ch], f32)
                    nc.vector.tensor_tensor(out=musq[:], in0=mu[:], in1=mu[:],
                                            op=ALU.mult)
                    var = bnp.tile([1, out_ch], f32)
                    nc.vector.tensor_tensor(out=var[:], in0=ex2[:],
                                            in1=musq[:], op=ALU.subtract)
                    sd = bnp.tile([1, out_ch], f32)
                    nc.scalar.activation(out=sd[:], in_=var[:], func=AF.Sqrt,
                                         bias=eps_c[0:1, 0:1], scale=1.0)
                    rs = bnp.tile([1, out_ch], f32)
                    nc.vector.reciprocal(out=rs[:], in_=sd[:])
                    scale_r = bnp.tile([1, out_ch], f32)
                    nc.vector.tensor_tensor(out=scale_r[:], in0=rs[:],
                                            in1=gam[:], op=ALU.mult)
                    mshift = bnp.tile([1, out_ch], f32)
                    nc.vector.tensor_tensor(out=mshift[:], in0=mu[:],
                                            in1=scale_r[:], op=ALU.mult)
                    shift_r = bnp.tile([1, out_ch], f32)
                    nc.vector.tensor_tensor(out=shift_r[:], in0=bet[:],
                                            in1=mshift[:], op=ALU.subtract)
                    scsh = bnp.tile([1, 2 * out_ch], f16)
                    nc.vector.tensor_copy(out=scsh[:, 0:out_ch],
                                          in_=scale_r[:])
                    nc.vector.tensor_copy(out=scsh[:, out_ch:], in_=shift_r[:])
                    bc_ps = bnps.tile([P, 2 * out_ch], f32)
                    nc.tensor.matmul(out=bc_ps[:], lhsT=ones1h[:],
                                     rhs=scsh[:], start=True, stop=True)
                    scale_bc = cs.tile([P, out_ch], f32)
                    nc.vector.tensor_copy(out=scale_bc[:],
                                          in_=bc_ps[:, 0:out_ch])
                    shift_bc = cs.tile([P, out_ch], f32)
                    nc.scalar.copy(out=shift_bc[:], in_=bc_ps[:, out_ch:])

                # ---------------- phase 3: normalize ----------------
                with tc.tile_pool(name="p3", bufs=3) as p3:
                    for t in range(n_tiles):
                        d0 = t * P
                        pt = min(P, shard - d0)
                        y_t = p3.tile([P, out_ch], f32, tag="y3")
                        nc.vector.tensor_tensor(out=y_t[:pt],
                                                in0=o_all[:pt, t, :],
                                                in1=scale_bc[:pt],
                                                op=ALU.mult)
                        nc.vector.tensor_tensor(out=y_t[:pt], in0=y_t[:pt],
                                                in1=shift_bc[:pt], op=ALU.add)
                        z_t = p3.tile([P, out_ch], f32, tag="z3")
                        nc.scalar.activation(out=z_t[:pt], in_=y_t[:pt],
                                             func=AF.Prelu,
                                             alpha=alpha_c[:pt, 0:1])
                        nc.sync.dma_start(out=out_fin[d0:d0 + pt, :],
                                          in_=z_t[:pt])

    nc.compile()
    return nc


def run(x, edge_index, W, att_src, att_dst, bias, gamma, beta,
        n_cores=8, trace=False, tmpdir=None):
    n_nodes, in_ch = x.shape
    shard = n_nodes // n_cores

    src = np.asarray(edge_index[0], np.int64)
    dst = np.asarray(edge_index[1], np.int64)
    sched, per_core = build_schedule(src, dst, n_nodes, n_cores)
    nc = build_program(sched)

    x = np.asarray(x, np.float32)
    iota_np = np.tile(np.arange(sched["cap_max"], dtype=np.float16), (P, 1))
    in_maps = []
    for c in range(n_cores):
        in_maps.append({
            "x": np.roll(x, -c * shard, axis=0).astype(np.float16),
            "w": np.asarray(W, np.float32),
            "att_src": np.asarray(att_src, np.float32),
            "att_dst": np.asarray(att_dst, np.float32),
            "gamma": np.asarray(gamma, np.float32).reshape(1, -1),
            "beta": np.asarray(beta, np.float32).reshape(1, -1),
            "g_idx": per_core[c]["g_idx"],
            "g_dl": per_core[c]["g_dl"],
            "iota": iota_np,
            "ident": np.eye(P, dtype=np.float32),
        })
    res = run_bass_kernel_spmd(nc, in_maps, core_ids=list(range(n_cores)),
                               trace=trace, tmpdir=tmpdir)
    out = np.concatenate([res.results[c]["out"] for c in range(n_cores)],
                         axis=0)
    return out, res


def _install_ntff_hook():
    """Best-effort NTFF profile hook for trace mode (missing in this image)."""
    import sys, types
    try:
        import antenv.axon_hooks  # noqa: F401
        return
    except ImportError:
        pass
    try:
        from trn_agent_boot.trn_boot import _ntff_profile_via_ctypes
        hook = _ntff_profile_via_ctypes('/opt/axon/libaxon_pjrt.so')
    except Exception:
        hook = None
    mod = types.ModuleType("antenv.axon_hooks")
    mod.get_axon_ntff_profile_hook = lambda: hook
    mod.set_axon_ntff_profile_hook = lambda h: None
    sys.modules["antenv.axon_hooks"] = mod


def kernel(**inputs):
    x = np.asarray(inputs["x"], np.float32)
    edge_index = np.asarray(inputs["edge_index"])
    W = np.asarray(inputs["W"], np.float32)
    att_src = np.asarray(inputs["att_src"], np.float32)
    att_dst = np.asarray(inputs["att_dst"], np.float32)
    bias = np.asarray(inputs["bias"], np.float32)
    gamma = np.asarray(inputs["gamma"], np.float32)
    beta = np.asarray(inputs["beta"], np.float32)
    import os
    trace = bool(os.environ.get("GAT_TRACE"))
    if trace:
        _install_ntff_hook()
    tmpdir = os.environ.get("GAT_TRACE_DIR")
    out, res = run(x, edge_index, W, att_src, att_dst, bias, gamma, beta,
                   trace=trace, tmpdir=tmpdir)
    kernel.last_result = res
    return out
